# revision 2
# baseline (speedup 1.0000x reference)
"""NativeFP4Linear TRN2 kernel: out = x @ (dequant(weight_fp4)).T + bias.

dequant(W)[o, i] = W[o, i] / block_scales[o*256 + i//16] / tensor_scale

Strategy (8 NeuronCores, tensor-parallel over out_features, 512 rows/core):
  - Host: transpose each core's weight slice to [in=4096, out=512] (matmul
    contracts along the partition dim, so the weight must be partition=i).
  - Device per core:
      rec   = 1/block_scales  (DVE reciprocal_approx_fast, [128, 1024] layout)
      rec   -> hi + lo fp32r pieces (exact sum)
      ES    = one-hot fp32r matmuls broadcast rec rows into [128 i, 512 o]
              expanded-scale tiles (hi+lo accumulated -> bit-exact fp32 scales)
      wdeq  = wT * ES           (DVE tensor-tensor, fp32 -> fp32r)
      out  += xT_chunk.T @ wdeq (fp32r matmuls, K accumulated in PSUM fp32)
      out   = out * (1/tensor_scale) + bias
  - Host: concatenate the 8 [32, 512] results -> [32, 4096].
"""
import numpy as np
from contextlib import ExitStack

import concourse.bass as bass
import concourse.mybir as mybir
import concourse.tile as tile
from concourse import bacc
from concourse.bass_utils import run_bass_kernel_spmd

F32 = mybir.dt.float32
F32R = mybir.dt.float32r

N_CORES = 8
B = 32            # batch
I = 4096          # in_features
O = 4096          # out_features
OC = O // N_CORES  # out features per core = 512
BS = 16           # fp4 block size
NBLK = I // BS    # block-columns per output row = 256
NSUB = I // 128   # 128-row contraction sub-chunks = 32
SUB_PER_IT = 3    # sub-chunks fused per pipeline iteration

_CACHE = {}


def _build():
    nc = bacc.Bacc("TRN2", target_bir_lowering=False, debug=False,
                   enable_asserts=True, num_devices=N_CORES)

    wt = nc.dram_tensor("wt", [I, OC], F32, kind="ExternalInput").ap()
    xt = nc.dram_tensor("xt", [128, NSUB * B], F32, kind="ExternalInput").ap()
    sc = nc.dram_tensor("sc", [128, NBLK * OC // 128], F32, kind="ExternalInput").ap()
    e16 = nc.dram_tensor("e16", [128, 16 * 128], F32R, kind="ExternalInput").ap()
    biasb = nc.dram_tensor("biasb", [B, OC], F32, kind="ExternalInput").ap()
    invts = nc.dram_tensor("invts", [B, 1], F32, kind="ExternalInput").ap()
    out = nc.dram_tensor("out", [B, OC], F32, kind="ExternalOutput").ap()

    with tile.TileContext(nc) as tc, ExitStack() as ctx:
        cpool = ctx.enter_context(tc.tile_pool(name="const", bufs=1))
        wpool = ctx.enter_context(tc.tile_pool(name="w", bufs=3))
        dqpool = ctx.enter_context(tc.tile_pool(name="dq", bufs=3))
        espool = ctx.enter_context(tc.tile_pool(name="es", bufs=2, space="PSUM"))
        mpool = ctx.enter_context(tc.tile_pool(name="acc", bufs=1, space="PSUM"))

        # ---- constants / setup ----
        t_xt = cpool.tile([128, NSUB * B], F32)
        nc.sync.dma_start(t_xt[:], xt[:])
        t_xtr = cpool.tile([128, NSUB * B], F32R)
        nc.vector.tensor_copy(t_xtr[:], t_xt[:])

        t_e16 = cpool.tile([128, 16 * 128], F32R)
        nc.sync.dma_start(t_e16[:], e16[:])

        t_sc = cpool.tile([128, 1024], F32)
        nc.sync.dma_start(t_sc[:], sc[:])
        t_rec = cpool.tile([128, 1024], F32)
        nc.vector.reciprocal_approx_fast(t_rec[:], t_sc[:])
        t_rhi = cpool.tile([128, 1024], F32R)
        nc.vector.tensor_copy(t_rhi[:], t_rec[:])
        t_rlo = cpool.tile([128, 1024], F32R)
        nc.vector.tensor_sub(t_rlo[:], t_rec[:], t_rhi[:].bitcast(F32))

        t_biasb = cpool.tile([B, OC], F32)
        nc.sync.dma_start(t_biasb[:], biasb[:])
        t_invts = cpool.tile([B, 1], F32)
        nc.sync.dma_start(t_invts[:], invts[:])

        t_acc = mpool.tile([B, OC], F32)

        # ---- main pipeline over contraction sub-chunks ----
        g = 0
        while g < NSUB:
            nsc = min(SUB_PER_IT, NSUB - g)
            rows = nsc * 128
            t_w = wpool.tile([128, SUB_PER_IT * OC], F32, tag="w")
            src = wt[g * 128:g * 128 + rows, :].rearrange(
                "(q p) n -> p q n", p=128)
            dst = t_w[:, :nsc * OC].rearrange("p (q n) -> p q n", q=nsc)
            nc.sync.dma_start(dst, src)

            t_es = espool.tile([128, SUB_PER_IT * OC], F32, tag="es")
            for j in range(nsc):
                gg = g + j
                v, u = gg % 16, gg // 16
                lhs = t_e16[:, 128 * v:128 * (v + 1)]
                dst = t_es[:, OC * j:OC * (j + 1)]
                nc.tensor.matmul(dst, lhs, t_rhi[:, OC * u:OC * (u + 1)],
                                 start=True, stop=False)
                nc.tensor.matmul(dst, lhs, t_rlo[:, OC * u:OC * (u + 1)],
                                 start=False, stop=True)

            t_dq = dqpool.tile([128, SUB_PER_IT * OC], F32R, tag="dq")
            nc.vector.tensor_mul(t_dq[:, :nsc * OC], t_w[:, :nsc * OC],
                                 t_es[:, :nsc * OC])

            for j in range(nsc):
                gg = g + j
                nc.tensor.matmul(t_acc[:], t_xtr[:, B * gg:B * (gg + 1)],
                                 t_dq[:, OC * j:OC * (j + 1)],
                                 start=(gg == 0), stop=(gg == NSUB - 1))
            g += nsc

        # ---- epilogue: out = acc * (1/ts) + bias ----
        t_out = cpool.tile([B, OC], F32)
        nc.vector.scalar_tensor_tensor(
            t_out[:], t_acc[:], t_invts[:], t_biasb[:],
            op0=mybir.AluOpType.mult, op1=mybir.AluOpType.add)
        nc.sync.dma_start(out[:], t_out[:])

    nc.compile()
    return nc


def _host_prep(x, weight_fp4, tensor_scale, block_scales, bias):
    """Build the per-core input maps."""
    x = np.asarray(x, dtype=np.float32)
    weight_fp4 = np.asarray(weight_fp4, dtype=np.float32)
    block_scales = np.asarray(block_scales, dtype=np.float32)
    bias = np.asarray(bias, dtype=np.float32)
    inv_ts = np.full((B, 1), 1.0 / float(np.asarray(tensor_scale).reshape(-1)[0]),
                     dtype=np.float32)

    # x.T tiled: xt[p, 32 g + b] = x[b, 128 g + p]
    xt = np.ascontiguousarray(
        x.T.reshape(NSUB, 128, B).transpose(1, 0, 2).reshape(128, NSUB * B))

    # one-hot selectors: e16[k, 128 v + p] = (k == 8 v + p // 16)
    e16 = np.zeros((128, 16 * 128), dtype=np.float32)
    k = np.arange(128)
    for v in range(16):
        p = np.arange(128)
        e16[8 * v + p // 16, 128 * v + p] = 1.0
    assert e16.sum() == 16 * 128

    bs2 = block_scales.reshape(O, NBLK)

    in_maps = []
    for c in range(N_CORES):
        o0 = c * OC
        wt_c = np.ascontiguousarray(weight_fp4[o0:o0 + OC, :].T)
        s_core = bs2[o0:o0 + OC, :].T  # [256 blk, 512 o]
        sc_c = np.ascontiguousarray(
            s_core.reshape(2, 128, OC).transpose(1, 0, 2).reshape(128, 1024))
        biasb_c = np.ascontiguousarray(
            np.broadcast_to(bias[o0:o0 + OC][None, :], (B, OC)))
        in_maps.append({
            "wt": wt_c, "xt": xt, "sc": sc_c, "e16": e16,
            "biasb": biasb_c, "invts": inv_ts,
        })
    return in_maps


def _get_program():
    if "nc" not in _CACHE:
        _CACHE["nc"] = _build()
    return _CACHE["nc"]


def kernel(x, weight_fp4, tensor_scale, block_scales, bias, **run_kwargs):
    nc = _get_program()
    in_maps = _host_prep(x, weight_fp4, tensor_scale, block_scales, bias)
    res = run_bass_kernel_spmd(nc, in_maps, core_ids=list(range(N_CORES)),
                               **run_kwargs)
    out = np.empty((B, O), dtype=np.float32)
    for c in range(N_CORES):
        out[:, c * OC:(c + 1) * OC] = res.results[c]["out"]
    if run_kwargs.get("trace"):
        kernel.last_exec_time_ns = res.exec_time_ns
    return out


# revision 3
# speedup vs baseline: 1.0618x; 1.0618x over previous
"""NativeFP4Linear TRN2 kernel: out = x @ (dequant(weight_fp4)).T + bias.

dequant(W)[o, i] = W[o, i] / block_scales[o*256 + i//16] / tensor_scale

Strategy (8 NeuronCores, tensor-parallel over out_features, 512 rows/core):
  - Host: transpose each core's weight slice to [in=4096, out=512] (matmul
    contracts along the partition dim, so the weight must be partition=i).
  - Device per core:
      rec   = 1/block_scales  (DVE reciprocal_approx_fast, [128, 1024] layout)
      rec   -> hi + lo fp32r pieces (exact sum)
      ES    = one-hot fp32r matmuls broadcast rec rows into [128 i, 512 o]
              expanded-scale tiles (hi+lo accumulated -> bit-exact fp32 scales)
      wdeq  = wT * ES           (DVE tensor-tensor, fp32 -> fp32r)
      out  += xT_chunk.T @ wdeq (fp32r matmuls, K accumulated in PSUM fp32)
      out   = out * (1/tensor_scale) + bias
  - Host: concatenate the 8 [32, 512] results -> [32, 4096].
"""
import numpy as np
from contextlib import ExitStack

import concourse.bass as bass
import concourse.mybir as mybir
import concourse.tile as tile
from concourse import bacc
from concourse.bass_utils import run_bass_kernel_spmd

F32 = mybir.dt.float32
F32R = mybir.dt.float32r

N_CORES = 8
B = 32            # batch
I = 4096          # in_features
O = 4096          # out_features
OC = O // N_CORES  # out features per core = 512
BS = 16           # fp4 block size
NBLK = I // BS    # block-columns per output row = 256
NSUB = I // 128   # 128-row contraction sub-chunks = 32
SUB_PER_IT = 3    # sub-chunks fused per pipeline iteration

_CACHE = {}


def _build():
    nc = bacc.Bacc("TRN2", target_bir_lowering=False, debug=False,
                   enable_asserts=True, num_devices=N_CORES)

    wt = nc.dram_tensor("wt", [I, OC], F32, kind="ExternalInput").ap()
    xt = nc.dram_tensor("xt", [128, NSUB * B], F32, kind="ExternalInput").ap()
    sc = nc.dram_tensor("sc", [128, NBLK * OC // 128], F32, kind="ExternalInput").ap()
    e16 = nc.dram_tensor("e16", [128, 16 * 128], F32R, kind="ExternalInput").ap()
    biasb = nc.dram_tensor("biasb", [B, OC], F32, kind="ExternalInput").ap()
    invts = nc.dram_tensor("invts", [B, 1], F32, kind="ExternalInput").ap()
    out = nc.dram_tensor("out", [B, OC], F32, kind="ExternalOutput").ap()

    with tile.TileContext(nc) as tc, ExitStack() as ctx:
        cpool = ctx.enter_context(tc.tile_pool(name="const", bufs=1))
        wpool = ctx.enter_context(tc.tile_pool(name="w", bufs=4))
        dqpool = ctx.enter_context(tc.tile_pool(name="dq", bufs=3))
        espool = ctx.enter_context(tc.tile_pool(name="es", bufs=2, space="PSUM"))
        mpool = ctx.enter_context(tc.tile_pool(name="acc", bufs=1, space="PSUM"))

        # iteration ranges over contraction sub-chunks
        starts = list(range(0, NSUB, SUB_PER_IT))
        sizes = [min(SUB_PER_IT, NSUB - s) for s in starts]
        n_it = len(starts)

        # ---- weight DMAs first so the stream starts immediately ----
        w_tiles = []
        for t in range(min(2, n_it)):
            g, nsc = starts[t], sizes[t]
            t_w = wpool.tile([128, SUB_PER_IT * OC], F32, tag="w")
            src = wt[g * 128:g * 128 + nsc * 128, :].rearrange(
                "(q p) n -> p q n", p=128)
            nc.sync.dma_start(t_w[:, :nsc * OC].rearrange(
                "p (q n) -> p q n", q=nsc), src)
            w_tiles.append(t_w)

        # ---- setup ----
        t_sc = cpool.tile([128, 1024], F32)
        nc.sync.dma_start(t_sc[:], sc[:])
        t_xt = cpool.tile([128, NSUB * B], F32)
        nc.sync.dma_start(t_xt[:], xt[:])
        t_e16 = cpool.tile([128, 16 * 128], F32R)
        nc.sync.dma_start(t_e16[:], e16[:])
        t_biasb = cpool.tile([B, OC], F32)
        nc.sync.dma_start(t_biasb[:], biasb[:])
        t_invts = cpool.tile([B, 1], F32)
        nc.sync.dma_start(t_invts[:], invts[:])

        t_rec = cpool.tile([128, 1024], F32)
        nc.vector.reciprocal_approx_fast(t_rec[:], t_sc[:])
        t_rhi = cpool.tile([128, 1024], F32R)
        nc.vector.tensor_copy(t_rhi[:], t_rec[:])
        t_rlo = cpool.tile([128, 1024], F32R)
        nc.vector.tensor_sub(t_rlo[:], t_rec[:], t_rhi[:].bitcast(F32))
        t_xtr = cpool.tile([128, NSUB * B], F32R)
        nc.vector.tensor_copy(t_xtr[:], t_xt[:])

        t_acc = mpool.tile([B, OC], F32)

        def emit_es(t):
            g, nsc = starts[t], sizes[t]
            t_es = espool.tile([128, SUB_PER_IT * OC], F32, tag="es")
            for j in range(nsc):
                gg = g + j
                v, u = gg % 16, gg // 16
                lhs = t_e16[:, 128 * v:128 * (v + 1)]
                dst = t_es[:, OC * j:OC * (j + 1)]
                nc.tensor.matmul(dst, lhs, t_rhi[:, OC * u:OC * (u + 1)],
                                 start=True, stop=False)
                nc.tensor.matmul(dst, lhs, t_rlo[:, OC * u:OC * (u + 1)],
                                 start=False, stop=True)
            return t_es

        # ---- software-pipelined main loop ----
        # PE order: ES(t+1) is emitted before main(t) so the tensor engine
        # fills the DVE-dequant latency with the next chunk's expansion.
        es_tiles = {0: emit_es(0)}
        for t in range(n_it):
            g, nsc = starts[t], sizes[t]
            if t + 2 < n_it:
                gg, nn = starts[t + 2], sizes[t + 2]
                t_w = wpool.tile([128, SUB_PER_IT * OC], F32, tag="w")
                src = wt[gg * 128:gg * 128 + nn * 128, :].rearrange(
                    "(q p) n -> p q n", p=128)
                nc.sync.dma_start(t_w[:, :nn * OC].rearrange(
                    "p (q n) -> p q n", q=nn), src)
                w_tiles.append(t_w)
            if t + 1 < n_it:
                es_tiles[t + 1] = emit_es(t + 1)

            t_es = es_tiles.pop(t)
            t_w = w_tiles[t]
            t_dq = dqpool.tile([128, SUB_PER_IT * OC], F32R, tag="dq")
            nc.vector.tensor_mul(t_dq[:, :nsc * OC], t_w[:, :nsc * OC],
                                 t_es[:, :nsc * OC])

            for j in range(nsc):
                gg = g + j
                nc.tensor.matmul(t_acc[:], t_xtr[:, B * gg:B * (gg + 1)],
                                 t_dq[:, OC * j:OC * (j + 1)],
                                 start=(gg == 0), stop=(gg == NSUB - 1))

        # ---- epilogue: out = acc * (1/ts) + bias ----
        t_out = cpool.tile([B, OC], F32)
        nc.vector.scalar_tensor_tensor(
            t_out[:], t_acc[:], t_invts[:], t_biasb[:],
            op0=mybir.AluOpType.mult, op1=mybir.AluOpType.add)
        nc.sync.dma_start(out[:], t_out[:])

    nc.compile()
    return nc


def _host_prep(x, weight_fp4, tensor_scale, block_scales, bias):
    """Build the per-core input maps."""
    x = np.asarray(x, dtype=np.float32)
    weight_fp4 = np.asarray(weight_fp4, dtype=np.float32)
    block_scales = np.asarray(block_scales, dtype=np.float32)
    bias = np.asarray(bias, dtype=np.float32)
    inv_ts = np.full((B, 1), 1.0 / float(np.asarray(tensor_scale).reshape(-1)[0]),
                     dtype=np.float32)

    # x.T tiled: xt[p, 32 g + b] = x[b, 128 g + p]
    xt = np.ascontiguousarray(
        x.T.reshape(NSUB, 128, B).transpose(1, 0, 2).reshape(128, NSUB * B))

    # one-hot selectors: e16[k, 128 v + p] = (k == 8 v + p // 16)
    e16 = np.zeros((128, 16 * 128), dtype=np.float32)
    k = np.arange(128)
    for v in range(16):
        p = np.arange(128)
        e16[8 * v + p // 16, 128 * v + p] = 1.0
    assert e16.sum() == 16 * 128

    bs2 = block_scales.reshape(O, NBLK)

    in_maps = []
    for c in range(N_CORES):
        o0 = c * OC
        wt_c = np.ascontiguousarray(weight_fp4[o0:o0 + OC, :].T)
        s_core = bs2[o0:o0 + OC, :].T  # [256 blk, 512 o]
        sc_c = np.ascontiguousarray(
            s_core.reshape(2, 128, OC).transpose(1, 0, 2).reshape(128, 1024))
        biasb_c = np.ascontiguousarray(
            np.broadcast_to(bias[o0:o0 + OC][None, :], (B, OC)))
        in_maps.append({
            "wt": wt_c, "xt": xt, "sc": sc_c, "e16": e16,
            "biasb": biasb_c, "invts": inv_ts,
        })
    return in_maps


def _get_program():
    if "nc" not in _CACHE:
        _CACHE["nc"] = _build()
    return _CACHE["nc"]


def kernel(x, weight_fp4, tensor_scale, block_scales, bias, **run_kwargs):
    nc = _get_program()
    in_maps = _host_prep(x, weight_fp4, tensor_scale, block_scales, bias)
    res = run_bass_kernel_spmd(nc, in_maps, core_ids=list(range(N_CORES)),
                               **run_kwargs)
    out = np.empty((B, O), dtype=np.float32)
    for c in range(N_CORES):
        out[:, c * OC:(c + 1) * OC] = res.results[c]["out"]
    if run_kwargs.get("trace"):
        kernel.last_exec_time_ns = res.exec_time_ns
    return out


# revision 7
# speedup vs baseline: 1.0670x; 1.0048x over previous
"""NativeFP4Linear TRN2 kernel: out = x @ (dequant(weight_fp4)).T + bias.

dequant(W)[o, i] = W[o, i] / block_scales[o*256 + i//16] / tensor_scale

Strategy (8 NeuronCores, tensor-parallel over out_features, 512 rows/core):
  - Host: transpose each core's weight slice to [in=4096, out=512] (matmul
    contracts along the partition dim, so the weight must be partition=i).
  - Device per core:
      rec   = 1/block_scales  (DVE reciprocal_approx_fast, [128, 1024] layout)
      rec   -> hi + lo fp32r pieces (exact sum)
      ES    = one-hot fp32r matmuls broadcast rec rows into [128 i, 512 o]
              expanded-scale tiles (hi+lo accumulated -> bit-exact fp32 scales)
      wdeq  = wT * ES           (DVE tensor-tensor, fp32 -> fp32r)
      out  += xT_chunk.T @ wdeq (fp32r matmuls, K accumulated in PSUM fp32)
      out   = out * (1/tensor_scale) + bias
  - Host: concatenate the 8 [32, 512] results -> [32, 4096].
"""
import numpy as np
from contextlib import ExitStack

import concourse.bass as bass
import concourse.mybir as mybir
import concourse.tile as tile
from concourse import bacc
from concourse.bass_utils import run_bass_kernel_spmd

F32 = mybir.dt.float32
F32R = mybir.dt.float32r

N_CORES = 8
B = 32            # batch
I = 4096          # in_features
O = 4096          # out_features
OC = O // N_CORES  # out features per core = 512
BS = 16           # fp4 block size
NBLK = I // BS    # block-columns per output row = 256
NSUB = I // 128   # 128-row contraction sub-chunks = 32
SUB_PER_IT = 3    # sub-chunks fused per pipeline iteration

_CACHE = {}


def _build():
    nc = bacc.Bacc("TRN2", target_bir_lowering=False, debug=False,
                   enable_asserts=True, num_devices=N_CORES)

    wt = nc.dram_tensor("wt", [I, OC], F32, kind="ExternalInput").ap()
    xt = nc.dram_tensor("xt", [128, NSUB * B], F32, kind="ExternalInput").ap()
    sc = nc.dram_tensor("sc", [128, NBLK * OC // 128], F32, kind="ExternalInput").ap()
    e16 = nc.dram_tensor("e16", [128, 16 * 128], mybir.dt.bfloat16,
                         kind="ExternalInput").ap()
    biasb = nc.dram_tensor("biasb", [B, OC], F32, kind="ExternalInput").ap()
    invts = nc.dram_tensor("invts", [B, 1], F32, kind="ExternalInput").ap()
    out = nc.dram_tensor("out", [B, OC], F32, kind="ExternalOutput").ap()

    with tile.TileContext(nc) as tc, ExitStack() as ctx:
        cpool = ctx.enter_context(tc.tile_pool(name="const", bufs=1))
        wpool = ctx.enter_context(tc.tile_pool(name="w", bufs=5))
        dqpool = ctx.enter_context(tc.tile_pool(name="dq", bufs=3))
        espool = ctx.enter_context(tc.tile_pool(name="es", bufs=2, space="PSUM"))
        mpool = ctx.enter_context(tc.tile_pool(name="acc", bufs=1, space="PSUM"))

        # iteration ranges over contraction sub-chunks
        starts = list(range(0, NSUB, SUB_PER_IT))
        sizes = [min(SUB_PER_IT, NSUB - s) for s in starts]
        n_it = len(starts)

        PREFETCH = 3

        def dma_w(t):
            # weight DMAs ride the Scalar HWDGE ring so their issue cost
            # doesn't serialize against the setup DMAs on the Sync ring
            g, nsc = starts[t], sizes[t]
            t_w = wpool.tile([128, SUB_PER_IT * OC], F32, tag="w")
            src = wt[g * 128:g * 128 + nsc * 128, :].rearrange(
                "(q p) n -> p q n", p=128)
            nc.scalar.dma_start(t_w[:, :nsc * OC].rearrange(
                "p (q n) -> p q n", q=nsc), src)
            return t_w

        # ---- setup (sc first: it heads the reciprocal critical path) ----
        t_sc = cpool.tile([128, 1024], F32)
        nc.sync.dma_start(t_sc[:], sc[:])

        w_tiles = [dma_w(t) for t in range(min(PREFETCH, n_it))]

        t_e16 = cpool.tile([128, 16 * 128], F32R)
        nc.gpsimd.dma_start(t_e16[:], e16[:])
        t_xt = cpool.tile([128, NSUB * B], F32)
        nc.sync.dma_start(t_xt[:], xt[:])
        t_biasb = cpool.tile([B, OC], F32)
        nc.sync.dma_start(t_biasb[:], biasb[:])
        t_invts = cpool.tile([B, 1], F32)
        nc.sync.dma_start(t_invts[:], invts[:])

        t_rec = cpool.tile([128, 1024], F32)
        nc.vector.reciprocal_approx_fast(t_rec[:], t_sc[:])
        t_rhi = cpool.tile([128, 1024], F32R)
        nc.vector.tensor_copy(t_rhi[:], t_rec[:])
        t_rlo = cpool.tile([128, 1024], F32R)
        nc.vector.tensor_sub(t_rlo[:], t_rec[:], t_rhi[:].bitcast(F32))
        t_xtr = cpool.tile([128, NSUB * B], F32R)
        nc.vector.tensor_copy(t_xtr[:], t_xt[:])

        t_acc = mpool.tile([B, OC], F32)

        def emit_es(t):
            g, nsc = starts[t], sizes[t]
            t_es = espool.tile([128, SUB_PER_IT * OC], F32, tag="es")
            for j in range(nsc):
                gg = g + j
                v, u = gg % 16, gg // 16
                lhs = t_e16[:, 128 * v:128 * (v + 1)]
                dst = t_es[:, OC * j:OC * (j + 1)]
                nc.tensor.matmul(dst, lhs, t_rhi[:, OC * u:OC * (u + 1)],
                                 start=True, stop=False)
                nc.tensor.matmul(dst, lhs, t_rlo[:, OC * u:OC * (u + 1)],
                                 start=False, stop=True)
            return t_es

        # ---- software-pipelined main loop ----
        # PE order: ES(t+1) is emitted before main(t) so the tensor engine
        # fills the DVE-dequant latency with the next chunk's expansion.
        es_tiles = {0: emit_es(0)}
        for t in range(n_it):
            g, nsc = starts[t], sizes[t]
            if t + PREFETCH < n_it:
                w_tiles.append(dma_w(t + PREFETCH))
            if t + 1 < n_it:
                es_tiles[t + 1] = emit_es(t + 1)

            t_es = es_tiles.pop(t)
            t_w = w_tiles[t]
            t_dq = dqpool.tile([128, SUB_PER_IT * OC], F32R, tag="dq")
            nc.vector.tensor_mul(t_dq[:, :nsc * OC], t_w[:, :nsc * OC],
                                 t_es[:, :nsc * OC])

            for j in range(nsc):
                gg = g + j
                nc.tensor.matmul(t_acc[:], t_xtr[:, B * gg:B * (gg + 1)],
                                 t_dq[:, OC * j:OC * (j + 1)],
                                 start=(gg == 0), stop=(gg == NSUB - 1))

        # ---- epilogue: out = acc * (1/ts) + bias ----
        t_out = cpool.tile([B, OC], F32)
        nc.vector.scalar_tensor_tensor(
            t_out[:], t_acc[:], t_invts[:], t_biasb[:],
            op0=mybir.AluOpType.mult, op1=mybir.AluOpType.add)
        nc.sync.dma_start(out[:], t_out[:])

    nc.compile()
    return nc


def _host_prep(x, weight_fp4, tensor_scale, block_scales, bias):
    """Build the per-core input maps."""
    x = np.asarray(x, dtype=np.float32)
    weight_fp4 = np.asarray(weight_fp4, dtype=np.float32)
    block_scales = np.asarray(block_scales, dtype=np.float32)
    bias = np.asarray(bias, dtype=np.float32)
    inv_ts = np.full((B, 1), 1.0 / float(np.asarray(tensor_scale).reshape(-1)[0]),
                     dtype=np.float32)

    # x.T tiled: xt[p, 32 g + b] = x[b, 128 g + p]
    xt = np.ascontiguousarray(
        x.T.reshape(NSUB, 128, B).transpose(1, 0, 2).reshape(128, NSUB * B))

    # one-hot selectors: e16[k, 128 v + p] = (k == 8 v + p // 16)
    import ml_dtypes
    e16 = np.zeros((128, 16 * 128), dtype=ml_dtypes.bfloat16)
    for v in range(16):
        p = np.arange(128)
        e16[8 * v + p // 16, 128 * v + p] = 1.0

    bs2 = block_scales.reshape(O, NBLK)

    in_maps = []
    for c in range(N_CORES):
        o0 = c * OC
        wt_c = np.ascontiguousarray(weight_fp4[o0:o0 + OC, :].T)
        s_core = bs2[o0:o0 + OC, :].T  # [256 blk, 512 o]
        sc_c = np.ascontiguousarray(
            s_core.reshape(2, 128, OC).transpose(1, 0, 2).reshape(128, 1024))
        biasb_c = np.ascontiguousarray(
            np.broadcast_to(bias[o0:o0 + OC][None, :], (B, OC)))
        in_maps.append({
            "wt": wt_c, "xt": xt, "sc": sc_c, "e16": e16,
            "biasb": biasb_c, "invts": inv_ts,
        })
    return in_maps


def _get_program():
    if "nc" not in _CACHE:
        _CACHE["nc"] = _build()
    return _CACHE["nc"]


def kernel(x, weight_fp4, tensor_scale, block_scales, bias, **run_kwargs):
    nc = _get_program()
    in_maps = _host_prep(x, weight_fp4, tensor_scale, block_scales, bias)
    res = run_bass_kernel_spmd(nc, in_maps, core_ids=list(range(N_CORES)),
                               **run_kwargs)
    out = np.empty((B, O), dtype=np.float32)
    for c in range(N_CORES):
        out[:, c * OC:(c + 1) * OC] = res.results[c]["out"]
    if run_kwargs.get("trace"):
        kernel.last_exec_time_ns = res.exec_time_ns
    return out
